# revision 8
# baseline (speedup 1.0000x reference)
"""KAN layer (B=4096, I=O=1024, grid 8, order 3) as a single K=10240 matmul.

Math: the degree-3 uniform-knot B-spline basis N_l(x) is an exact linear
combination of 11 cheap features of x:
    [1, x, x^2, x^3, kink_k(x)^3 for the 7 interior knots]
where kink_k(x) = relu(x - c_k) for right-half knots and relu(c_k - x) for
left-half knots (mixed anchoring keeps every feature <= 1, which keeps the
change-of-basis well conditioned for the reduced-precision fp32r matmul).
The constant column folds into a bias; the x column also absorbs base_weight.
So   out = x @ Wb.T + einsum('bil,oil->bo', basis(x), Ws)
        == phi(x) @ Wt + bias            with K = 10 * 1024 columns.

Device work per core (batch-sharded, 512 rows):
  - 10 feature tiles per 128-wide i-chunk, computed on ACT/DVE (3 ops/knot)
  - fp32r matmuls accumulating all 80 K-chunks into 8 PSUM banks
  - bias add on PSUM evacuation
"""
import numpy as np

import concourse.bass as bass
import concourse.tile as tile
from concourse import bacc, mybir
from concourse import bass_utils

NCORES = 8
B, I, O = 4096, 1024, 1024
BC = B // NCORES  # 512 rows per core
SPLINE_ORDER = 3
NF = 10  # x, x^2, x^3, 7 kink^3   (constant column -> bias)
K = NF * I  # 10240
NK = K // 128  # 80
NMT = BC // 128  # 4 batch tiles per core

_USE_F32R = True  # fp32r: 1 cyc/row on PE vs 4 for fp32


def _bspline_basis_np(x, grid, k=SPLINE_ORDER):
    xe = x[..., None]
    basis = ((xe >= grid[:-1]) & (xe < grid[1:])).astype(np.float64)
    for d in range(1, k + 1):
        ld = grid[d:-1] - grid[: -(d + 1)]
        rd = grid[d + 1 :] - grid[1:-d]
        ld = np.where(ld > 0, ld, 1.0)
        rd = np.where(rd > 0, rd, 1.0)
        basis = (xe - grid[: -(d + 1)]) / ld * basis[..., :-1] + (
            grid[d + 1 :] - xe
        ) / rd * basis[..., 1:]
    return basis


def _features_np(x, kinks):
    cols = [x, x**2, x**3]
    for c, left in kinks:
        cols.append(np.maximum(c - x, 0.0) ** 3 if left else np.maximum(x - c, 0.0) ** 3)
    return np.stack(cols, axis=-1)


def _fold_weights(base_weight, spline_weight, grid):
    """-> Wt (K, O) float32 k=(m,i) m-major, bias (O,) float64, kinks list."""
    grid = np.asarray(grid, np.float64)
    kinks = [(float(c), bool(c < 0)) for c in grid if -1.0 + 1e-9 < c < 1.0 - 1e-9]
    assert len(kinks) == NF - 3, kinks
    xs = np.linspace(-1.0, 0.999999, 4001)
    Phi = np.concatenate([_features_np(xs, kinks), np.ones((len(xs), 1))], axis=-1)
    Bm = _bspline_basis_np(xs, grid)
    T, *_ = np.linalg.lstsq(Phi, Bm, rcond=None)  # (NF+1, 11)
    assert np.abs(Phi @ T - Bm).max() < 1e-8
    T = T.T  # (11, NF+1)
    ws = spline_weight.astype(np.float64)  # (O, I, 11)
    Wt0 = np.einsum("oil,lm->oim", ws, T[:, :NF])
    Wt0[:, :, 0] += base_weight.astype(np.float64)
    bias = (ws * T[:, NF][None, None, :]).sum(axis=(1, 2))  # (O,)
    Wt = np.ascontiguousarray(
        Wt0.transpose(2, 1, 0).reshape(K, O), dtype=np.float32
    )
    return Wt, bias, kinks


def _build_nc(kinks):
    fdt = mybir.dt.float32r if _USE_F32R else mybir.dt.float32
    f32 = mybir.dt.float32
    nc = bacc.Bacc("TRN2", target_bir_lowering=False, debug=False)
    xT_d = nc.dram_tensor("xT", [I // 128, 128, BC], fdt, kind="ExternalInput").ap()
    wq_d = nc.dram_tensor("wq", [NK, 128, O], fdt, kind="ExternalInput").ap()
    bias_d = nc.dram_tensor("biasrep", [128, O], f32, kind="ExternalInput").ap()
    ab_d = nc.dram_tensor("actbias", [128, NF - 3], f32, kind="ExternalInput").ap()
    out_d = nc.dram_tensor("out", [BC, O], f32, kind="ExternalOutput").ap()

    with tile.TileContext(nc) as tc:
        with (
            tc.tile_pool(name="xp", bufs=1) as xp,
            tc.tile_pool(name="wp", bufs=8) as wp,
            tc.tile_pool(name="rp", bufs=3) as rp,
            tc.tile_pool(name="tp", bufs=2) as tp,
            tc.tile_pool(name="op", bufs=3) as op,
            tc.tile_pool(name="pp", bufs=1, space=bass.MemorySpace.PSUM) as pp,
        ):
            # x/bias loads go on the gpsimd (SWDGE) queues so the weight
            # stream on sync (HWDGE) starts immediately
            btile = xp.tile([128, O], f32, name="btile")
            nc.gpsimd.dma_start(btile[:], bias_d[:])
            abt = xp.tile([128, NF - 3], f32, name="abt")
            nc.gpsimd.dma_start(abt[:], ab_d[:])
            xtr, x2r, x3r = [], [], []
            for ic in range(I // 128):
                t = xp.tile([128, BC], fdt, name=f"xtr{ic}")
                nc.gpsimd.dma_start(t[:], xT_d[ic])
                xtr.append(t)
            for ic in range(I // 128):
                t2 = xp.tile([128, BC], fdt, name=f"x2r{ic}")
                nc.vector.tensor_mul(t2[:], xtr[ic][:], xtr[ic][:])
                x2r.append(t2)
                t3 = xp.tile([128, BC], fdt, name=f"x3r{ic}")
                nc.vector.tensor_mul(t3[:], t2[:], xtr[ic][:])
                x3r.append(t3)
            ps = [
                pp.tile([128, 512], f32, name=f"ps{mt}_{h}")
                for mt in range(NMT)
                for h in range(2)
            ]
            for kc in range(NK):
                m, ic = divmod(kc, I // 128)
                wt = wp.tile([128, O], fdt, tag="w", name=f"w{kc}")
                nc.sync.dma_start(wt[:], wq_d[kc])
                if m == 0:
                    feat = xtr[ic]
                elif m == 1:
                    feat = x2r[ic]
                elif m == 2:
                    feat = x3r[ic]
                else:
                    c, left = kinks[m - 3]
                    r = tp.tile([128, BC], f32, tag="r", name=f"r{kc}")
                    nc.scalar.activation(
                        r[:],
                        xtr[ic][:],
                        mybir.ActivationFunctionType.Relu,
                        bias=abt[:, m - 3 : m - 2],
                        scale=(-1.0 if left else 1.0),
                    )
                    r2 = tp.tile([128, BC], f32, tag="r2", name=f"r2_{kc}")
                    nc.vector.tensor_mul(r2[:], r[:], r[:])
                    feat = rp.tile([128, BC], fdt, tag="r3", name=f"r3_{kc}")
                    nc.vector.tensor_mul(feat[:], r2[:], r[:])
                for mt in range(NMT):
                    for h in range(2):
                        nc.tensor.matmul(
                            ps[mt * 2 + h][:],
                            feat[:, 128 * mt : 128 * (mt + 1)],
                            wt[:, 512 * h : 512 * (h + 1)],
                            start=(kc == 0),
                            stop=(kc == NK - 1),
                        )
            for mt in range(NMT):
                for h in range(2):
                    so = op.tile([128, 512], f32, tag="so", name=f"so{mt}_{h}")
                    nc.vector.tensor_add(
                        so[:], ps[mt * 2 + h][:], btile[:, 512 * h : 512 * (h + 1)]
                    )
                    nc.sync.dma_start(
                        out_d[128 * mt : 128 * (mt + 1), 512 * h : 512 * (h + 1)],
                        so[:],
                    )
    nc.compile()
    return nc


_CACHE = {}


def kernel(x, base_weight, spline_weight, grid, _trace=False):
    x = np.ascontiguousarray(x, np.float32)
    Wt, bias, kinks = _fold_weights(base_weight, spline_weight, grid)
    wq = np.ascontiguousarray(Wt.reshape(NK, 128, O))
    biasrep = np.ascontiguousarray(
        np.broadcast_to(bias.astype(np.float32), (128, O))
    )
    actbias = np.ascontiguousarray(
        np.broadcast_to(
            np.array(
                [(c if left else -c) for c, left in kinks], np.float32
            ),
            (128, NF - 3),
        )
    )
    key = "nc"
    if key not in _CACHE:
        _CACHE[key] = _build_nc(kinks)
    nc = _CACHE[key]
    in_maps = []
    for c in range(NCORES):
        xT = np.ascontiguousarray(
            x[c * BC : (c + 1) * BC].T.reshape(I // 128, 128, BC)
        )
        in_maps.append({"xT": xT, "wq": wq, "biasrep": biasrep, "actbias": actbias})
    res = bass_utils.run_bass_kernel_spmd(
        nc, in_maps, core_ids=list(range(NCORES)), trace=_trace
    )
    out = np.concatenate([res.results[c]["out"] for c in range(NCORES)], axis=0)
    if _trace:
        kernel._last_result = res
    return out.astype(np.float32)


# revision 9
# speedup vs baseline: 1.0258x; 1.0258x over previous
"""KAN layer (B=4096, I=O=1024, grid 8, order 3) as a single K=10240 matmul.

Math: the degree-3 uniform-knot B-spline basis N_l(x) is an exact linear
combination of 11 cheap features of x:
    [1, x, x^2, x^3, kink_k(x)^3 for the 7 interior knots]
where kink_k(x) = relu(x - c_k) for right-half knots and relu(c_k - x) for
left-half knots (mixed anchoring keeps every feature <= 1, which keeps the
change-of-basis well conditioned for the reduced-precision fp32r matmul).
The constant column folds into a bias; the x column also absorbs base_weight.
So   out = x @ Wb.T + einsum('bil,oil->bo', basis(x), Ws)
        == phi(x) @ Wt + bias            with K = 10 * 1024 columns.

Device work per core (batch-sharded, 512 rows):
  - 10 feature tiles per 128-wide i-chunk, computed on ACT/DVE (3 ops/knot)
  - fp32r matmuls accumulating all 80 K-chunks into 8 PSUM banks
  - bias add on PSUM evacuation
"""
import numpy as np

import concourse.bass as bass
import concourse.tile as tile
from concourse import bacc, mybir
from concourse import bass_utils

NCORES = 8
B, I, O = 4096, 1024, 1024
BC = B // NCORES  # 512 rows per core
SPLINE_ORDER = 3
NF = 10  # x, x^2, x^3, 7 kink^3   (constant column -> bias)
K = NF * I  # 10240
NK = K // 128  # 80
NMT = BC // 128  # 4 batch tiles per core

_USE_F32R = True  # fp32r: 1 cyc/row on PE vs 4 for fp32


def _bspline_basis_np(x, grid, k=SPLINE_ORDER):
    xe = x[..., None]
    basis = ((xe >= grid[:-1]) & (xe < grid[1:])).astype(np.float64)
    for d in range(1, k + 1):
        ld = grid[d:-1] - grid[: -(d + 1)]
        rd = grid[d + 1 :] - grid[1:-d]
        ld = np.where(ld > 0, ld, 1.0)
        rd = np.where(rd > 0, rd, 1.0)
        basis = (xe - grid[: -(d + 1)]) / ld * basis[..., :-1] + (
            grid[d + 1 :] - xe
        ) / rd * basis[..., 1:]
    return basis


def _features_np(x, kinks):
    cols = [x, x**2, x**3]
    for c, left in kinks:
        cols.append(np.maximum(c - x, 0.0) ** 3 if left else np.maximum(x - c, 0.0) ** 3)
    return np.stack(cols, axis=-1)


def _fold_weights(base_weight, spline_weight, grid):
    """-> Wt (K, O) float32 k=(m,i) m-major, bias (O,) float64, kinks list."""
    grid = np.asarray(grid, np.float64)
    kinks = [(float(c), bool(c < 0)) for c in grid if -1.0 + 1e-9 < c < 1.0 - 1e-9]
    assert len(kinks) == NF - 3, kinks
    xs = np.linspace(-1.0, 0.999999, 4001)
    Phi = np.concatenate([_features_np(xs, kinks), np.ones((len(xs), 1))], axis=-1)
    Bm = _bspline_basis_np(xs, grid)
    T, *_ = np.linalg.lstsq(Phi, Bm, rcond=None)  # (NF+1, 11)
    assert np.abs(Phi @ T - Bm).max() < 1e-8
    T = T.T  # (11, NF+1)
    ws = spline_weight.astype(np.float64)  # (O, I, 11)
    Wt0 = np.einsum("oil,lm->oim", ws, T[:, :NF])
    Wt0[:, :, 0] += base_weight.astype(np.float64)
    bias = (ws * T[:, NF][None, None, :]).sum(axis=(1, 2))  # (O,)
    Wt = np.ascontiguousarray(
        Wt0.transpose(2, 1, 0).reshape(K, O), dtype=np.float32
    )
    return Wt, bias, kinks


def _build_nc(kinks):
    fdt = mybir.dt.float32r if _USE_F32R else mybir.dt.float32
    f32 = mybir.dt.float32
    nc = bacc.Bacc("TRN2", target_bir_lowering=False, debug=False)
    xT_d = nc.dram_tensor("xT", [I // 128, 128, BC], fdt, kind="ExternalInput").ap()
    wq_d = nc.dram_tensor("wq", [NK, 128, O], fdt, kind="ExternalInput").ap()
    bias_d = nc.dram_tensor("biasrep", [128, O], f32, kind="ExternalInput").ap()
    ab_d = nc.dram_tensor("actbias", [128, NF - 3], f32, kind="ExternalInput").ap()
    out_d = nc.dram_tensor("out", [BC, O], f32, kind="ExternalOutput").ap()

    with tile.TileContext(nc) as tc:
        with (
            tc.tile_pool(name="xp", bufs=1) as xp,
            tc.tile_pool(name="wp", bufs=8) as wp,
            tc.tile_pool(name="rp", bufs=3) as rp,
            tc.tile_pool(name="tp", bufs=2) as tp,
            tc.tile_pool(name="op", bufs=3) as op,
            tc.tile_pool(name="pp", bufs=1, space=bass.MemorySpace.PSUM) as pp,
        ):
            # x/bias loads go on the gpsimd (SWDGE) queues so the weight
            # stream on sync (HWDGE) starts immediately; bias after x
            xtr, x2r, x3r = [], [], []
            for ic in range(I // 128):
                t = xp.tile([128, BC], fdt, name=f"xtr{ic}")
                nc.gpsimd.dma_start(t[:], xT_d[ic])
                xtr.append(t)
            abt = xp.tile([128, NF - 3], f32, name="abt")
            nc.gpsimd.dma_start(abt[:], ab_d[:])
            btile = xp.tile([128, O], f32, name="btile")
            nc.gpsimd.dma_start(btile[:], bias_d[:])
            for ic in range(I // 128):
                t2 = xp.tile([128, BC], fdt, name=f"x2r{ic}")
                nc.vector.tensor_mul(t2[:], xtr[ic][:], xtr[ic][:])
                x2r.append(t2)
                t3 = xp.tile([128, BC], fdt, name=f"x3r{ic}")
                nc.vector.tensor_mul(t3[:], t2[:], xtr[ic][:])
                x3r.append(t3)
            ps = [
                pp.tile([128, 512], f32, name=f"ps{mt}_{h}")
                for mt in range(NMT)
                for h in range(2)
            ]
            for kc in range(NK):
                m, ic = divmod(kc, I // 128)
                wt = wp.tile([128, O], fdt, tag="w", name=f"w{kc}")
                nc.sync.dma_start(wt[:], wq_d[kc])
                if m == 0:
                    feat = xtr[ic]
                elif m == 1:
                    feat = x2r[ic]
                elif m == 2:
                    feat = x3r[ic]
                else:
                    c, left = kinks[m - 3]
                    r = tp.tile([128, BC], f32, tag="r", name=f"r{kc}")
                    nc.scalar.activation(
                        r[:],
                        xtr[ic][:],
                        mybir.ActivationFunctionType.Relu,
                        bias=abt[:, m - 3 : m - 2],
                        scale=(-1.0 if left else 1.0),
                    )
                    r2 = tp.tile([128, BC], f32, tag="r2", name=f"r2_{kc}")
                    nc.vector.tensor_mul(r2[:], r[:], r[:])
                    feat = rp.tile([128, BC], fdt, tag="r3", name=f"r3_{kc}")
                    nc.vector.tensor_mul(feat[:], r2[:], r[:])
                for mt in range(NMT):
                    for h in range(2):
                        nc.tensor.matmul(
                            ps[mt * 2 + h][:],
                            feat[:, 128 * mt : 128 * (mt + 1)],
                            wt[:, 512 * h : 512 * (h + 1)],
                            start=(kc == 0),
                            stop=(kc == NK - 1),
                        )
            for mt in range(NMT):
                for h in range(2):
                    so = op.tile([128, 512], f32, tag="so", name=f"so{mt}_{h}")
                    nc.vector.tensor_add(
                        so[:], ps[mt * 2 + h][:], btile[:, 512 * h : 512 * (h + 1)]
                    )
                    nc.sync.dma_start(
                        out_d[128 * mt : 128 * (mt + 1), 512 * h : 512 * (h + 1)],
                        so[:],
                    )
    nc.compile()
    return nc


_CACHE = {}


def kernel(x, base_weight, spline_weight, grid, _trace=False):
    x = np.ascontiguousarray(x, np.float32)
    Wt, bias, kinks = _fold_weights(base_weight, spline_weight, grid)
    wq = np.ascontiguousarray(Wt.reshape(NK, 128, O))
    biasrep = np.ascontiguousarray(
        np.broadcast_to(bias.astype(np.float32), (128, O))
    )
    actbias = np.ascontiguousarray(
        np.broadcast_to(
            np.array(
                [(c if left else -c) for c, left in kinks], np.float32
            ),
            (128, NF - 3),
        )
    )
    key = "nc"
    if key not in _CACHE:
        _CACHE[key] = _build_nc(kinks)
    nc = _CACHE[key]
    in_maps = []
    for c in range(NCORES):
        xT = np.ascontiguousarray(
            x[c * BC : (c + 1) * BC].T.reshape(I // 128, 128, BC)
        )
        in_maps.append({"xT": xT, "wq": wq, "biasrep": biasrep, "actbias": actbias})
    res = bass_utils.run_bass_kernel_spmd(
        nc, in_maps, core_ids=list(range(NCORES)), trace=_trace
    )
    out = np.concatenate([res.results[c]["out"] for c in range(NCORES)], axis=0)
    if _trace:
        kernel._last_result = res
    return out.astype(np.float32)


# revision 12
# speedup vs baseline: 1.0271x; 1.0012x over previous
"""KAN layer (B=4096, I=O=1024, grid 8, order 3) as a single K=10240 matmul.

Math: the degree-3 uniform-knot B-spline basis N_l(x) is an exact linear
combination of 11 cheap features of x:
    [1, x, x^2, x^3, kink_k(x)^3 for the 7 interior knots]
where kink_k(x) = relu(x - c_k) for right-half knots and relu(c_k - x) for
left-half knots (mixed anchoring keeps every feature <= 1, which keeps the
change-of-basis well conditioned for the reduced-precision fp32r matmul).
The constant column folds into a bias; the x column also absorbs base_weight.
So   out = x @ Wb.T + einsum('bil,oil->bo', basis(x), Ws)
        == phi(x) @ Wt + bias            with K = 10 * 1024 columns.

Device work per core (batch-sharded, 512 rows):
  - 10 feature tiles per 128-wide i-chunk, computed on ACT/DVE (3 ops/knot)
  - fp32r matmuls accumulating all 80 K-chunks into 8 PSUM banks
  - bias add on PSUM evacuation
"""
import numpy as np

import concourse.bass as bass
import concourse.tile as tile
from concourse import bacc, mybir
from concourse import bass_utils

NCORES = 8
B, I, O = 4096, 1024, 1024
BC = B // NCORES  # 512 rows per core
SPLINE_ORDER = 3
NF = 10  # x, x^2, x^3, 7 kink^3   (constant column -> bias)
K = NF * I  # 10240
NK = K // 128  # 80
NMT = BC // 128  # 4 batch tiles per core

_USE_F32R = True  # fp32r: 1 cyc/row on PE vs 4 for fp32


def _bspline_basis_np(x, grid, k=SPLINE_ORDER):
    xe = x[..., None]
    basis = ((xe >= grid[:-1]) & (xe < grid[1:])).astype(np.float64)
    for d in range(1, k + 1):
        ld = grid[d:-1] - grid[: -(d + 1)]
        rd = grid[d + 1 :] - grid[1:-d]
        ld = np.where(ld > 0, ld, 1.0)
        rd = np.where(rd > 0, rd, 1.0)
        basis = (xe - grid[: -(d + 1)]) / ld * basis[..., :-1] + (
            grid[d + 1 :] - xe
        ) / rd * basis[..., 1:]
    return basis


def _features_np(x, kinks):
    cols = [x, x**2, x**3]
    for c, left in kinks:
        cols.append(np.maximum(c - x, 0.0) ** 3 if left else np.maximum(x - c, 0.0) ** 3)
    return np.stack(cols, axis=-1)


def _fold_weights(base_weight, spline_weight, grid):
    """-> Wt (K, O) float32 k=(m,i) m-major, bias (O,) float64, kinks list."""
    grid = np.asarray(grid, np.float64)
    kinks = [(float(c), bool(c < 0)) for c in grid if -1.0 + 1e-9 < c < 1.0 - 1e-9]
    assert len(kinks) == NF - 3, kinks
    xs = np.linspace(-1.0, 0.999999, 4001)
    Phi = np.concatenate([_features_np(xs, kinks), np.ones((len(xs), 1))], axis=-1)
    Bm = _bspline_basis_np(xs, grid)
    T, *_ = np.linalg.lstsq(Phi, Bm, rcond=None)  # (NF+1, 11)
    assert np.abs(Phi @ T - Bm).max() < 1e-8
    T = T.T  # (11, NF+1)
    ws = spline_weight.astype(np.float64)  # (O, I, 11)
    Wt0 = np.einsum("oil,lm->oim", ws, T[:, :NF])
    Wt0[:, :, 0] += base_weight.astype(np.float64)
    bias = (ws * T[:, NF][None, None, :]).sum(axis=(1, 2))  # (O,)
    Wt = np.ascontiguousarray(
        Wt0.transpose(2, 1, 0).reshape(K, O), dtype=np.float32
    )
    return Wt, bias, kinks


def _build_nc(kinks):
    fdt = mybir.dt.float32r if _USE_F32R else mybir.dt.float32
    f32 = mybir.dt.float32
    nc = bacc.Bacc("TRN2", target_bir_lowering=False, debug=False)
    xT_d = nc.dram_tensor("xT", [I // 128, 128, BC], fdt, kind="ExternalInput").ap()
    wq_d = nc.dram_tensor("wq", [NK, 128, O], fdt, kind="ExternalInput").ap()
    bias_d = nc.dram_tensor("biasrep", [128, O], f32, kind="ExternalInput").ap()
    ab_d = nc.dram_tensor("actbias", [128, NF - 3], f32, kind="ExternalInput").ap()
    out_d = nc.dram_tensor("out", [BC, O], f32, kind="ExternalOutput").ap()

    with tile.TileContext(nc) as tc:
        with (
            tc.tile_pool(name="xp", bufs=1) as xp,
            tc.tile_pool(name="wp", bufs=8) as wp,
            tc.tile_pool(name="rp", bufs=6) as rp,
            tc.tile_pool(name="tp", bufs=4) as tp,
            tc.tile_pool(name="op", bufs=3) as op,
            tc.tile_pool(name="pp", bufs=1, space=bass.MemorySpace.PSUM) as pp,
        ):
            # x/bias loads go on the gpsimd (SWDGE) queues so the weight
            # stream on sync (HWDGE) starts immediately; bias after x
            xtr, x2r, x3r = [], [], []
            for ic in range(I // 128):
                t = xp.tile([128, BC], fdt, name=f"xtr{ic}")
                nc.gpsimd.dma_start(t[:], xT_d[ic])
                xtr.append(t)
            abt = xp.tile([128, NF - 3], f32, name="abt")
            nc.gpsimd.dma_start(abt[:], ab_d[:])
            btile = xp.tile([128, O], f32, name="btile")
            nc.gpsimd.dma_start(btile[:], bias_d[:])
            for ic in range(I // 128):
                t2 = xp.tile([128, BC], fdt, name=f"x2r{ic}")
                nc.scalar.activation(
                    t2[:], xtr[ic][:], mybir.ActivationFunctionType.Square
                )
                x2r.append(t2)
                t3 = xp.tile([128, BC], fdt, name=f"x3r{ic}")
                nc.vector.tensor_mul(t3[:], t2[:], xtr[ic][:])
                x3r.append(t3)
            ps = [
                pp.tile([128, 512], f32, name=f"ps{mt}_{h}")
                for mt in range(NMT)
                for h in range(2)
            ]
            for kc in range(NK):
                m, ic = divmod(kc, I // 128)
                wt = wp.tile([128, O], fdt, tag="w", name=f"w{kc}")
                nc.sync.dma_start(wt[:], wq_d[kc])
                if m == 0:
                    feat = xtr[ic]
                elif m == 1:
                    feat = x2r[ic]
                elif m == 2:
                    feat = x3r[ic]
                else:
                    c, left = kinks[m - 3]
                    r = tp.tile([128, BC], f32, tag="r", name=f"r{kc}")
                    nc.scalar.activation(
                        r[:],
                        xtr[ic][:],
                        mybir.ActivationFunctionType.Relu,
                        bias=abt[:, m - 3 : m - 2],
                        scale=(-1.0 if left else 1.0),
                    )
                    r2 = tp.tile([128, BC], f32, tag="r2", name=f"r2_{kc}")
                    if kc % 2 == 0:
                        # balance the cube chain: ACT does half the squares
                        nc.scalar.activation(
                            r2[:], r[:], mybir.ActivationFunctionType.Square
                        )
                    else:
                        nc.vector.tensor_mul(r2[:], r[:], r[:])
                    feat = rp.tile([128, BC], fdt, tag="r3", name=f"r3_{kc}")
                    nc.vector.tensor_mul(feat[:], r2[:], r[:])
                for mt in range(NMT):
                    for h in range(2):
                        nc.tensor.matmul(
                            ps[mt * 2 + h][:],
                            feat[:, 128 * mt : 128 * (mt + 1)],
                            wt[:, 512 * h : 512 * (h + 1)],
                            start=(kc == 0),
                            stop=(kc == NK - 1),
                        )
            for mt in range(NMT):
                for h in range(2):
                    so = op.tile([128, 512], f32, tag="so", name=f"so{mt}_{h}")
                    nc.vector.tensor_add(
                        so[:], ps[mt * 2 + h][:], btile[:, 512 * h : 512 * (h + 1)]
                    )
                    nc.sync.dma_start(
                        out_d[128 * mt : 128 * (mt + 1), 512 * h : 512 * (h + 1)],
                        so[:],
                    )
    nc.compile()
    return nc


_CACHE = {}


def kernel(x, base_weight, spline_weight, grid, _trace=False):
    x = np.ascontiguousarray(x, np.float32)
    Wt, bias, kinks = _fold_weights(base_weight, spline_weight, grid)
    wq = np.ascontiguousarray(Wt.reshape(NK, 128, O))
    biasrep = np.ascontiguousarray(
        np.broadcast_to(bias.astype(np.float32), (128, O))
    )
    actbias = np.ascontiguousarray(
        np.broadcast_to(
            np.array(
                [(c if left else -c) for c, left in kinks], np.float32
            ),
            (128, NF - 3),
        )
    )
    key = "nc"
    if key not in _CACHE:
        _CACHE[key] = _build_nc(kinks)
    nc = _CACHE[key]
    in_maps = []
    for c in range(NCORES):
        xT = np.ascontiguousarray(
            x[c * BC : (c + 1) * BC].T.reshape(I // 128, 128, BC)
        )
        in_maps.append({"xT": xT, "wq": wq, "biasrep": biasrep, "actbias": actbias})
    res = bass_utils.run_bass_kernel_spmd(
        nc, in_maps, core_ids=list(range(NCORES)), trace=_trace
    )
    out = np.concatenate([res.results[c]["out"] for c in range(NCORES)], axis=0)
    if _trace:
        kernel._last_result = res
    return out.astype(np.float32)
